# revision 40
# baseline (speedup 1.0000x reference)
import sys

sys.path.insert(0, "/opt/trn_rl_repo")

import numpy as np

D_MODEL = 1024
NUM_HEADS = 16
HEAD_DIM = 64
B = 2
S = 2048
N_CORES = 8
HG = 4          # head-groups (cores per batch)
HPC = 4         # heads per core
DL = 256        # local feature width per core (HPC * HEAD_DIM)

_cache = {}
last_exec_time_ns = None

DRIP = 4        # filler instructions interleaved per attention m-step


def _build(has_qkvb):
    import concourse.bacc as bacc
    import concourse.mybir as mybir
    import concourse.tile as tile

    F32 = mybir.dt.float32
    F32R = mybir.dt.float32r
    BF16 = mybir.dt.bfloat16
    Exp = mybir.ActivationFunctionType.Exp
    mult = mybir.AluOpType.mult
    is_ge = mybir.AluOpType.is_ge

    nc = bacc.Bacc("TRN2", target_bir_lowering=False, debug=False)
    # packed layouts: x as [128 part, n-chunk, i-chunk, 512] and wq as
    # [128 part, slab-group, i-chunk, 256] so every DMA chunk is contiguous
    # per partition (2KB+ lines -> full burst bandwidth).
    xT_d = nc.dram_tensor("xTh", (128, 4, 8, 512), BF16, kind="ExternalInput")
    wq_d = nc.dram_tensor("wqh", (128, 3, 8, 256), BF16, kind="ExternalInput")
    wo_d = nc.dram_tensor("woh", (128, 2, D_MODEL), BF16, kind="ExternalInput")
    if has_qkvb:
        qb_d = nc.dram_tensor("qb", (1, 3, 256), BF16, kind="ExternalInput")
    ident_d = nc.dram_tensor("ident", (128, 128), BF16, kind="ExternalInput")
    tneg_d = nc.dram_tensor("tneg", (128, 128), BF16, kind="ExternalInput")
    out_d = nc.dram_tensor("out", (S, D_MODEL), BF16, kind="ExternalOutput")

    with tile.TileContext(nc) as tc:
        with tc.tile_pool(name="persist", bufs=1) as persist, \
             tc.tile_pool(name="work", bufs=1) as work, \
             tc.tile_pool(name="pmm", bufs=1, space="PSUM") as pmm:

            xtb = persist.tile([128, 4, 8, 512], BF16, name="xtb")
            wqb = persist.tile([128, 3, 8, 256], BF16, name="wqb")
            wob = persist.tile([128, 2, D_MODEL], BF16, name="wob")
            # Q/K packed per head-pair p: partitions 0:64 head 2p, 64:128 head 2p+1
            QT = [persist.tile([128, S], BF16, name=f"QT{p}") for p in range(2)]
            KT = [persist.tile([128, S], BF16, name=f"KT{p}") for p in range(2)]
            # V augmented [pair, parity, key-tile, 128]: per key-tile block:
            # [V dims 64 | ones 64]; partitions of block st = keys of tile st
            Vaug5 = persist.tile([128, 2, 2, 16, 128], BF16, name="Vaug")
            ctxp = [persist.tile([128, S], BF16, name=f"ctxp{p}") for p in range(2)]
            identb = persist.tile([128, 128], BF16, name="identb")
            tnegb = persist.tile([128, 128], BF16, name="tnegb")

            # DMAs in need order, split over TWO trigger queues (each
            # DMA_DIRECT2D costs ~0.6-1.3us of issue time on its engine
            # queue, so a single queue serializes the start). Weights go on
            # sync, x slices on the otherwise-idle gpsimd queue. wq columns
            # are host-reordered as [Q-pair0 | K-pair0 | V | Q-pair1 |
            # K-pair1] so the critical pair-0 QK weights land first.
            nc.sync.dma_start(out=wqb[:, 0, :, :], in_=wq_d[:, 0, :, :])
            nc.gpsimd.dma_start(out=xtb[:, 0, 0:4, :], in_=xT_d[:, 0, 0:4, :])
            nc.gpsimd.dma_start(out=xtb[:, 0, 4:8, :], in_=xT_d[:, 0, 4:8, :])
            nc.sync.dma_start(out=wqb[:, 1, :, :], in_=wq_d[:, 1, :, :])
            nc.sync.dma_start(out=identb[:], in_=ident_d[:])
            nc.sync.dma_start(out=tnegb[:], in_=tneg_d[:])
            if has_qkvb:
                qb_t = persist.tile([1, 3, 256], BF16, name="qb_t")
                nc.sync.dma_start(out=qb_t[:], in_=qb_d[:])
                ones_t = persist.tile([1, 512], BF16, name="ones_t")
                nc.vector.memset(ones_t[:], 1.0)
            nc.sync.dma_start(out=wqb[:, 2, :, :], in_=wq_d[:, 2, :, :])
            nc.sync.dma_start(out=wob[:], in_=wo_d[:])
            nc.gpsimd.dma_start(out=xtb[:, 1, :, :], in_=xT_d[:, 1, :, :])
            nc.gpsimd.dma_start(out=xtb[:, 2, :, :], in_=xT_d[:, 2, :, :])
            nc.gpsimd.dma_start(out=xtb[:, 3, :, :], in_=xT_d[:, 3, :, :])

            # causal masking is folded into the scores matmul: the band tile
            # accumulates ident.T @ tneg, where tneg[k, c] = -240 for c < k.
            # exp(0.125 * (s - 240)) ~ 3e-14 zeroes the dead region. Both
            # constants come from the host (DMA'd first, above).
            # Only the ones-halves of Vaug blocks need init (V cols are
            # overwritten by the projection copies).
            for pr in range(2):
                nc.vector.memset(Vaug5[:, pr, :, 0:4, 64:128], 1.0)

            # ---- projection / out-projection instruction generators ----
            # mi -> (slab group, col offset) in the packed wq layout
            MIGRP = {0: (0, 0), 2: (0, 128), 1: (2, 0), 3: (2, 128)}

            def qk_items(mi, n):
                # psq = sum_i wq[i][:, mi-block].T @ xt[i][:, n-chunk]  -> [128 feat, 512 seq]
                g, off = MIGRP[mi]
                items = []
                st = {}

                def mk(i):
                    def f():
                        if i == 0:
                            st['ps'] = pmm.tile([128, 512], F32, tag="pp", bufs=2, name="psq")
                        nc.tensor.matmul(
                            out=st['ps'][:],
                            lhsT=wqb[:, g, i, off:off + 128],
                            rhs=xtb[:, n, i, :],
                            start=(i == 0),
                            stop=(i == 7 and not has_qkvb),
                        )
                    return f
                for i in range(8):
                    items.append(mk(i))
                if has_qkvb:
                    def fb():
                        nc.tensor.matmul(
                            out=st['ps'][:],
                            lhsT=qb_t[0:1, g, off:off + 128],
                            rhs=ones_t[0:1, :],
                            start=False, stop=True,
                        )
                    items.append(fb)

                def cp():
                    dst = QT[mi] if mi < 2 else KT[mi - 2]
                    nc.vector.tensor_copy(
                        out=dst[:, 512 * n:512 * (n + 1)], in_=st['ps'][:])
                items.append(cp)
                return items

            def v_items(sti):
                # psv = sum_i xt[i][:, st-block].T @ wq[i][:, V cols] -> [128 seq, 256 feat]
                items = []
                st = {}

                def mk(i):
                    def f():
                        if i == 0:
                            st['ps'] = pmm.tile([128, 2, 2, 64], F32, tag="pp", bufs=2, name="psv")
                        nc.tensor.matmul(
                            out=st['ps'][:],
                            lhsT=xtb[:, sti // 4, i, 128 * (sti % 4):128 * (sti % 4 + 1)],
                            rhs=wqb[:, 1, i, :],
                            start=(i == 0),
                            stop=(i == 7 and not has_qkvb),
                        )
                    return f
                for i in range(8):
                    items.append(mk(i))
                if has_qkvb:
                    def fb():
                        nc.tensor.matmul(
                            out=st['ps'][:],
                            lhsT=ones_t[0:1, 0:128],
                            rhs=qb_t[0:1, 1, :],
                            start=False, stop=True,
                        )
                    items.append(fb)

                def cp():
                    nc.vector.tensor_copy(
                        out=Vaug5[:, :, :, sti, 0:64],
                        in_=st['ps'][:, :, :, :])
                items.append(cp)
                return items

            def outproj_items(qm, act_half=False, wide=False):
                items = []
                st = {}

                def half(n):
                    def f():
                        if wide:
                            if n == 0:
                                st['pw'] = pmm.tile([128, 2, 512], F32, tag="s", bufs=2, name="psow")
                            ps = st['pw'][:, n, :]
                        else:
                            ps = pmm.tile([128, 512], F32, tag="pp", bufs=2, name="pso")[:]
                        nc.tensor.matmul(
                            out=ps,
                            lhsT=ctxp[0][:, 128 * qm:128 * (qm + 1)],
                            rhs=wob[:, 0, 512 * n:512 * (n + 1)],
                            start=True, stop=False,
                        )
                        nc.tensor.matmul(
                            out=ps,
                            lhsT=ctxp[1][:, 128 * qm:128 * (qm + 1)],
                            rhs=wob[:, 1, 512 * n:512 * (n + 1)],
                            start=False, stop=True,
                        )
                        if n == 0:
                            st['stage'] = work.tile([128, D_MODEL], BF16, tag="st", bufs=3, name="stage")
                        if wide:
                            if n == 1:
                                nc.vector.tensor_copy(
                                    out=st['stage'][:, 0:1024],
                                    in_=st['pw'][:, :, :])
                        elif act_half:
                            # tail chunks: stage copies on the (idle) scalar
                            # engine so the DVE queue stays off the critical
                            # path
                            nc.scalar.copy(
                                out=st['stage'][:, 512 * n:512 * (n + 1)], in_=ps)
                        else:
                            nc.vector.tensor_copy(
                                out=st['stage'][:, 512 * n:512 * (n + 1)], in_=ps)
                    return f
                items.append(half(0))
                items.append(half(1))

                def dm():
                    eng = nc.sync if qm % 2 == 0 else nc.gpsimd
                    eng.dma_start(out=out_d[128 * qm:128 * (qm + 1), :], in_=st['stage'][:])
                items.append(dm)
                return items

            # ---- attention scores issue ----
            # both heads' K=64 matmuls are issued adjacently: their lhsT
            # base partitions (0 / 64) map to different PE row groups, so
            # they run CONCURRENTLY. The K=128 band (mask) matmuls go after
            # so they don't serialize the pair.
            def issue_scores(p, j, m):
                t = m - 4 * j
                lo = 128 * t if t > 0 else 0
                band = t >= 0
                ps = pmm.tile([128, 2, 512], F32, tag="s", bufs=2, name="psS")
                for h, pr in ((0, slice(0, 64)), (1, slice(64, 128))):
                    nc.tensor.matmul(
                        out=ps[:, h, lo:512],
                        lhsT=KT[p][pr, 128 * m:128 * (m + 1)],
                        rhs=QT[p][pr, 512 * j + lo:512 * (j + 1)],
                        start=True, stop=not band,
                    )
                if band:
                    for h in (0, 1):
                        nc.tensor.matmul(
                            out=ps[:, h, lo:lo + 128],
                            lhsT=identb[:],
                            rhs=tnegb[:],
                            start=False, stop=True,
                        )
                return ps

            # ---- HAM warm-up: dummy matmuls on a scratch tile keep the PE
            # busy during the initial DMA wait so the clock gate opens
            # (K=8/8, 2.4 GHz) before the first real matmul.
            warm = persist.tile([128, 640], BF16, name="warm")
            nc.vector.memset(warm[:], 0.0)
            psd = pmm.tile([128, 512], F32, tag="pp", bufs=2, name="psd")
            for _ in range(10):
                nc.tensor.matmul(
                    out=psd[:], lhsT=warm[:, 0:128], rhs=warm[:, 128:640],
                    start=True, stop=True)

            # ---- immediate emission: minimum needed for attn(0, 0) ----
            # qk(0,0) and qk(2,0) interleaved per i so each matmul runs as its
            # x/w slice lands
            psq0 = pmm.tile([128, 512], F32, tag="pp", bufs=2, name="psq0")
            psq2 = pmm.tile([128, 512], F32, tag="pp", bufs=2, name="psq2")
            for i in range(8):
                for mi, pst in ((0, psq0), (2, psq2)):
                    g, off = MIGRP[mi]
                    nc.tensor.matmul(
                        out=pst[:],
                        lhsT=wqb[:, g, i, off:off + 128],
                        rhs=xtb[:, 0, i, :],
                        start=(i == 0),
                        stop=(i == 7 and not has_qkvb),
                    )
            if has_qkvb:
                for mi, pst in ((0, psq0), (2, psq2)):
                    g, off = MIGRP[mi]
                    nc.tensor.matmul(
                        out=pst[:],
                        lhsT=qb_t[0:1, g, off:off + 128],
                        rhs=ones_t[0:1, :],
                        start=False, stop=True,
                    )
            nc.vector.tensor_copy(out=QT[0][:, 0:512], in_=psq0[:])
            nc.vector.tensor_copy(out=KT[0][:, 0:512], in_=psq2[:])
            psprev = issue_scores(0, 0, 0)
            for sti in range(4):
                for it in v_items(sti):
                    it()

            # ---- filler queue for the rest, drained during attention ----
            # blocks are processed pair-interleaved ascending j, so every
            # prerequisite is emitted just-in-time: block (p,j) only needs
            # its own Q chunk, K chunks n<=j and V tiles sti<=4j+3.
            FQ = []
            need_idx = {(0, 0): 0}
            need_v = {sti: 0 for sti in range(4)}
            FQ += qk_items(1, 0) + qk_items(3, 0)
            need_idx[(1, 0)] = len(FQ)
            for n in range(1, 4):
                for pr in range(2):
                    FQ.append(lambda n=n, pr=pr: nc.vector.memset(
                        Vaug5[:, pr, :, 4 * n:4 * (n + 1), 64:128], 1.0))
                FQ += qk_items(0, n) + qk_items(2, n)
                need_idx[(0, n)] = len(FQ)
                # V tiles drip during block (0,n) itself, gated per AV step
                # by need_v — defers ~4us of projection work per n into the
                # exp-bound block interior.
                for sti in range(4 * n, 4 * n + 4):
                    FQ += v_items(sti)
                    need_v[sti] = len(FQ)
                FQ += qk_items(1, n) + qk_items(3, n)
                need_idx[(1, n)] = len(FQ)

            drained = [0]

            def drain_to(k):
                while drained[0] < k:
                    FQ[drained[0]]()
                    drained[0] += 1

            def drip(r):
                drain_to(min(drained[0] + r, len(FQ)))

            def norm_cols(p, j, psA, asl, psB, bsl, cols):
                # normalize ctxp[p][:, 512j + cols] — even-head data in
                # psA[asl], odd-head in psB[bsl] (each [128, w]: ctx rows
                # 0:64, sums rows 64:128)
                w = cols.stop - cols.start
                dst = slice(512 * j + cols.start, 512 * j + cols.stop)
                sumsE = work.tile([64, 512], F32, tag="sE", bufs=2, name="sumsE")
                nc.vector.tensor_copy(out=sumsE[:, 0:w], in_=psA[64:128, asl])
                recE = work.tile([64, 512], F32, tag="rE", bufs=2, name="recE")
                nc.vector.reciprocal_approx_fast(recE[:, 0:w], sumsE[:, 0:w])
                nc.vector.tensor_tensor(
                    out=ctxp[p][0:64, dst],
                    in0=psA[0:64, asl], in1=recE[:, 0:w], op=mult)
                sumsO = work.tile([64, 512], F32, tag="sO", bufs=2, name="sumsO")
                nc.vector.tensor_copy(out=sumsO[:, 0:w], in_=psB[64:128, bsl])
                recO = work.tile([64, 512], F32, tag="rO", bufs=2, name="recO")
                nc.vector.reciprocal_approx_fast(recO[:, 0:w], sumsO[:, 0:w])
                codd = work.tile([64, 512], BF16, tag="cO", bufs=2, name="codd")
                nc.vector.tensor_tensor(
                    out=codd[:, 0:w], in0=psB[0:64, bsl], in1=recO[:, 0:w], op=mult)
                nc.vector.tensor_copy(
                    out=ctxp[p][64:128, dst], in_=codd[:, 0:w])

            blocks = [(0, 0), (1, 0), (0, 1), (1, 1), (0, 2), (1, 2), (0, 3), (1, 3)]
            drip_rate = {}
            for bi, (p, j) in enumerate(blocks):
                last = bi == len(blocks) - 1
                split = j == 3
                drain_to(need_idx[(p, j)])
                rate = drip_rate.get((p, j), DRIP)
                mlast = 4 * j + 3
                psA = pmm.tile([128, 512], F32, tag="a", bufs=1, name="psA")
                psB = pmm.tile([128, 512], F32, tag="b", bufs=1, name="psB")
                # software pipeline: iteration k emits scores(k+1), exp(k),
                # then AV(k-1) — AV trails exp by a full step so the tensor
                # engine never waits on the activation engine
                prev = None
                for k in range(mlast + 2):
                    if k <= mlast:
                        ps = psprev
                        if k < mlast:
                            psprev = issue_scores(p, j, k + 1)
                        elif bi + 1 < len(blocks):
                            # RAW safety: next block's Q/K producer copies must
                            # be emitted before any instruction reading them
                            drain_to(need_idx[blocks[bi + 1]])
                            psprev = issue_scores(*blocks[bi + 1], 0)
                        t = k - 4 * j
                        w0 = 128 * t if t > 0 else 0
                        e = work.tile([128, 2, 512], BF16, tag="e", bufs=4, name="e")
                        nc.scalar.activation(
                            e[:, :, w0:512], ps[:, :, w0:512], Exp, scale=0.125)
                        cur = (e, k, w0)
                    else:
                        cur = None
                    if prev is not None:
                        e_, m_, w0_ = prev
                        if p == 0 and m_ >= 4:
                            # RAW: AV(m) reads Vaug tile m — force its
                            # projection items out before the matmul
                            drain_to(need_v[m_])
                        if not split:
                            nc.tensor.matmul(
                                out=psA[:, w0_:512],
                                lhsT=Vaug5[:, p, 0, m_, :],
                                rhs=e_[:, 0, w0_:512],
                                start=(m_ == 0), stop=(m_ == mlast),
                            )
                            nc.tensor.matmul(
                                out=psB[:, w0_:512],
                                lhsT=Vaug5[:, p, 1, m_, :],
                                rhs=e_[:, 1, w0_:512],
                                start=(m_ == 0), stop=(m_ == mlast),
                            )
                        else:
                            # last block: AV split by query-column half —
                            # cols 0:256 of both parities go to psA (last
                            # write at m=13), cols 256:512 to psB (m=15).
                            # psA's norm + out-proj then overlap the
                            # block's remaining steps without PE-W/DVE-R
                            # on the same bank.
                            # start=True clears has_written for the WHOLE
                            # bank, so only the first (par=0) matmul per
                            # bank may carry it; par=1's first write relies
                            # on flags=0 overwrite-where-unwritten.
                            for par in (0, 1):
                                if w0_ < 256:
                                    nc.tensor.matmul(
                                        out=psA[:, 256 * par + w0_:256 * (par + 1)],
                                        lhsT=Vaug5[:, p, par, m_, :],
                                        rhs=e_[:, par, w0_:256],
                                        start=(m_ == 0 and par == 0),
                                        stop=(m_ == 13),
                                        skip_group_check=True,
                                    )
                                hi = max(w0_, 256)
                                nc.tensor.matmul(
                                    out=psB[:, 256 * par + hi - 256:256 * (par + 1)],
                                    lhsT=Vaug5[:, p, par, m_, :],
                                    rhs=e_[:, par, hi:512],
                                    start=(m_ == 0 and par == 0),
                                    stop=(m_ == mlast),
                                    skip_group_check=True,
                                )
                            if m_ == 13:
                                norm_cols(p, j, psA, slice(0, 256),
                                          psA, slice(256, 512), slice(0, 256))
                            elif m_ == mlast and last:
                                for qm in (4 * j, 4 * j + 1):
                                    for it in outproj_items(qm, act_half=True):
                                        it()
                    prev = cur
                    if k < mlast + 1:
                        drip(rate)
                if not split:
                    norm_cols(p, j, psA, slice(0, 512),
                              psB, slice(0, 512), slice(0, 512))
                else:
                    norm_cols(p, j, psB, slice(0, 256),
                              psB, slice(256, 512), slice(256, 512))
                drip(8)
                if last:
                    for qm in (4 * j + 2, 4 * j + 3):
                        for it in outproj_items(qm, act_half=True):
                            it()
                elif p == 1:
                    for qm in range(4 * j, 4 * j + 4):
                        FQ += outproj_items(qm)
            drain_to(len(FQ))

    nc.finalize()
    return nc


def _ident():
    from ml_dtypes import bfloat16
    return np.eye(128, dtype=np.float32).astype(bfloat16)


def _tneg():
    from ml_dtypes import bfloat16
    k = np.arange(128)[:, None]
    c = np.arange(128)[None, :]
    return np.where(c < k, np.float32(-240.0), np.float32(0.0)).astype(bfloat16)


def kernel(x, qkv_w, qkv_b, out_w, out_b):
    from concourse import bass_utils
    from ml_dtypes import bfloat16
    global last_exec_time_ns

    x = np.ascontiguousarray(np.asarray(x, dtype=np.float32))
    qkv_w = np.asarray(qkv_w, dtype=np.float32)
    qkv_b = np.asarray(qkv_b, dtype=np.float32)
    out_w = np.asarray(out_w, dtype=np.float32)
    out_b = np.asarray(out_b, dtype=np.float32)

    has_qkvb = bool(np.any(qkv_b))
    if has_qkvb not in _cache:
        _cache[has_qkvb] = _build(has_qkvb)
    nc = _cache[has_qkvb]

    in_maps = []
    for c in range(N_CORES):
        b, hg = divmod(c, HG)
        xT = x[b].T.astype(bfloat16)                       # [1024, 2048]
        # [128 part, n-chunk 4, i-chunk 8, 512] — contiguous 8KB/partition
        # per n-chunk DMA
        xTh = np.ascontiguousarray(
            xT.reshape(8, 128, 4, 512).transpose(1, 2, 0, 3))
        rows = np.concatenate([
            qkv_w[DL * hg:DL * (hg + 1)],
            qkv_w[D_MODEL + DL * hg:D_MODEL + DL * (hg + 1)],
            qkv_w[2 * D_MODEL + DL * hg:2 * D_MODEL + DL * (hg + 1)],
        ], axis=0)
        wqT = rows.T.astype(bfloat16)                      # [1024, 768]
        # column order [Q-pair0|K-pair0 | V | Q-pair1|K-pair1] (see MIGRP),
        # then packed [128 part, slab-group 3, i-chunk 8, 256]
        wqT = np.concatenate([
            wqT[:, 0:128], wqT[:, 256:384], wqT[:, 512:768],
            wqT[:, 128:256], wqT[:, 384:512]], axis=1)
        wqh = np.ascontiguousarray(
            wqT.reshape(8, 128, 3, 256).transpose(1, 2, 0, 3))
        woT = out_w[:, DL * hg:DL * (hg + 1)].T.astype(bfloat16)  # [256, 1024]
        woh = np.ascontiguousarray(
            woT.reshape(2, 128, D_MODEL).transpose(1, 0, 2))
        m = {"xTh": xTh, "wqh": wqh, "woh": woh,
             "ident": _ident(), "tneg": _tneg()}
        if has_qkvb:
            qb = np.concatenate([
                qkv_b[DL * hg:DL * (hg + 1)],
                qkv_b[D_MODEL + DL * hg:D_MODEL + DL * (hg + 1)],
                qkv_b[2 * D_MODEL + DL * hg:2 * D_MODEL + DL * (hg + 1)],
            ])
            qb = np.concatenate([
                qb[0:128], qb[256:384], qb[512:768], qb[128:256], qb[384:512]])
            m["qb"] = qb.reshape(1, 3, 256).astype(bfloat16)
        in_maps.append(m)

    res = bass_utils.run_bass_kernel_spmd(nc, in_maps, core_ids=list(range(N_CORES)))
    last_exec_time_ns = res.exec_time_ns

    out = np.zeros((B, S, D_MODEL), dtype=np.float32)
    for c in range(N_CORES):
        b, hg = divmod(c, HG)
        out[b] += res.results[c]["out"].astype(np.float32)
    out += out_b[None, None, :]
    return out

